# revision 1
# baseline (speedup 1.0000x reference)
"""Trainium2 Bass kernel for a dense multi-head attention layer.

Reference computation (per batch b):
    qkv = x @ w_qkv.T + b_qkv                # [L, 3H]
    q, k, v per head (NH=16 heads, HD=64)
    attn = softmax((q @ k.T) * HD**-0.5)
    out  = (attn @ v) per head, concat, @ w_out.T + b_out

Sharding across 8 NeuronCores: core c handles batch b = c // 4 and the
4-head group g = c % 4 (heads 4g .. 4g+3, organized as 2 pairs of 2).
Each core computes its partial output projection [L, H]; the host sums
the 4 partials per batch and adds b_out.

Per-core on-device plan:
  - All 16-bit operand paths use fp16 (values are small, so fp16's extra
    mantissa beats bf16); q/k/scores stay fp32.  Matmul operand pairs are
    width-uniform (neuronxcc rejects 32-bit x 16-bit mixes).
  - x streams in as fp16 chunk tiles; weights fp16; a handful of big
    DMAs in dependency order (each DMA pays a ~625ns serialized HWDGE
    hold, so fewer + bigger wins the startup race).
  - qT/kT are [HD, L] per head (head pairs stacked on 128 partitions,
    fp32r); v is produced per 128-key tile in natural [keys, HD] fp16
    layout augmented with a ones column (so attn @ v_aug also yields the
    softmax denominator z).
  - Scores are computed transposed, ST[j, i] (keys on partitions), one
    128-key tile at a time, two heads row-tiled on the PE (K=64 each).
  - exp() on the Scalar engine with the 1/8 scale fused; max-subtraction
    is skipped (|scores/8| <= ~6, safely inside exp range).
  - PV runs TRANSPOSED: out[q, d] with queries on partitions, using pt
    128-query slices as the stationary operand and v_aug [128, 65] fp16
    moving -- 65-row matmuls at full rate, half the PE cycles of the
    [65, 512] orientation; z lands in column 64 with q on partitions so
    normalization is a native per-partition tensor_scalar on DVE.
    PV trails exp by one step so chunk-transition PSUM WAR waits hide
    behind the exp stream instead of stalling the in-order PE queue.
  - o2 [q, d] flips back to [d, q] for the output projection via a
    DMA-engine transpose (InstDmaTransposeAnt) -- zero PE/DVE cost.
  - Stores are full [128, 1024] fp16 rows (halves the serialized HWDGE
    holds); a tiny-matmul warmup stream holds the PE at its 2.4GHz
    p-state through the initial DMA window.
  - Fillers are globally load-balanced: the first chunk only carries its
    own head-pair's v-chains plus the pinned k-chains; pair-1 v-chains,
    later q/k projections and the output projection ride the exp slack of
    the ACT-bound middle chunks (every middle chunk stays under the
    16.6us exp pace).  Scores for step jt+1 are emitted ahead of exp(jt),
    carrying across chunk boundaries (st_in/nxt handover).
"""

import sys

sys.path.insert(0, "/opt/trn_rl_repo")

import numpy as np

H = 1024
NH = 16
HD = 64
L = 2048
B = 2
N_CORES = 8
HEADS_PER_CORE = 4
KT = H // 128  # 8 k-tiles over the hidden dim
IC = L // 512  # 4 i-chunks of 512 queries
JT = L // 128  # 16 j-tiles of 128 keys

_CACHE = {}


def _build():
    import concourse.bass as bass
    import concourse.mybir as mybir
    import concourse.tile as tile
    from concourse import bacc

    F32 = mybir.dt.float32
    F32R = mybir.dt.float32r
    F16 = mybir.dt.float16
    EXP = mybir.ActivationFunctionType.Exp

    nc = bacc.Bacc("TRN2", target_bir_lowering=False, debug=False,
                   num_devices=N_CORES)

    xT_d = nc.declare_dram_parameter("xT", [H, L], F16, isOutput=False)
    wqT_d = nc.declare_dram_parameter("wqT", [128, 2048], F16, isOutput=False)
    wkT_d = nc.declare_dram_parameter("wkT", [128, 2048], F16, isOutput=False)
    wvT_d = nc.declare_dram_parameter("wvT", [128, 2048], F16, isOutput=False)
    woutT_d = nc.declare_dram_parameter("woutT", [256, H], F16, isOutput=False)
    bqk_d = nc.declare_dram_parameter("bqk", [128, 4], F32, isOutput=False)
    bvrep_d = nc.declare_dram_parameter("bvrep", [128, 256], F32, isOutput=False)
    ident_d = nc.declare_dram_parameter("ident", [128, 128], F16, isOutput=False)
    out_d = nc.declare_dram_parameter("out", [L, H], F16, isOutput=True)

    with tile.TileContext(nc) as tc, nc.allow_low_precision(
            reason="fp16 operand tiles; all reductions accumulate in fp32 "
                   "PSUM"):
        with tc.tile_pool(name="sbW", bufs=1) as sbW, \
             tc.tile_pool(name="sbA", bufs=1) as sbA, \
             tc.tile_pool(name="sbPT", bufs=4) as sbPT, \
             tc.tile_pool(name="sbZ", bufs=2) as sbZ, \
             tc.tile_pool(name="sbOut", bufs=4) as sbOut, \
             tc.tile_pool(name="psST", bufs=2, space="PSUM") as psST, \
             tc.tile_pool(name="psACC", bufs=1, space="PSUM") as psACC, \
             tc.tile_pool(name="psCH", bufs=2, space="PSUM") as psCH:

            # ---- resident loads: few big DMAs, dependency order ----------
            # Transfer order IS the critical path: wq then x chunk 0 then wk
            # gets the q-chain going at ~6us.
            xT_r = xT_d.rearrange("(k p) n -> p k n", p=128)
            # First-use tensors stream in halves (k-tiles 0-3, then 4-7) so
            # the q/k chains can start ~2us earlier.
            # wq/wk are pair-major [p, pair, k, d]: the first chains need
            # only pair-0's half, and the partition-major DRAM layout keeps
            # every DMA run at 2KB (no sub-512B descriptor penalty).
            wq = sbW.tile([128, 2, KT, 128], F16, tag="wq", name="wq")
            wk = sbW.tile([128, 2, KT, 128], F16, tag="wk", name="wk")
            x0 = sbW.tile([128, KT, 512], F16, tag="x0", name="x0")
            nc.sync.dma_start(out=x0[:, 0:4, :], in_=xT_r[:, 0:4, 0:512])
            nc.sync.dma_start(
                out=wq[:, 0, :, :],
                in_=wqT_d[:, 0:1024].rearrange("p (k d) -> p k d", k=KT))
            nc.sync.dma_start(
                out=wk[:, 0, :, :],
                in_=wkT_d[:, 0:1024].rearrange("p (k d) -> p k d", k=KT))
            nc.sync.dma_start(out=x0[:, 4:8, :], in_=xT_r[:, 4:8, 0:512])
            bqk_sb = sbW.tile([128, 4], F32)
            nc.sync.dma_start(out=bqk_sb, in_=bqk_d[:, :])
            wv = sbW.tile([128, 2, KT, 128], F16, tag="wv", name="wv")
            nc.sync.dma_start(
                out=wv[:, 0, :, :],
                in_=wvT_d[:, 0:1024].rearrange("p (k d) -> p k d", k=KT))
            xt = [x0]
            x1 = sbW.tile([128, KT, 512], F16, tag="x1", name="x1")
            nc.sync.dma_start(out=x1, in_=xT_r[:, :, 512:1024])
            xt.append(x1)
            # x2/x3 feed seg0 fillers at ~18us; the pair-1 weights are not
            # needed until ~35us -- keep x ahead of them in the DMA queue
            bvrep = sbW.tile([128, 256], F32)
            nc.sync.dma_start(out=bvrep, in_=bvrep_d[:, :])
            for c in range(2, IC):
                xc = sbW.tile([128, KT, 512], F16, tag=f"x{c}", name=f"x{c}")
                nc.sync.dma_start(out=xc, in_=xT_r[:, :, 512 * c:512 * (c + 1)])
                xt.append(xc)
            nc.sync.dma_start(
                out=wq[:, 1, :, :],
                in_=wqT_d[:, 1024:2048].rearrange("p (k d) -> p k d", k=KT))
            nc.sync.dma_start(
                out=wk[:, 1, :, :],
                in_=wkT_d[:, 1024:2048].rearrange("p (k d) -> p k d", k=KT))
            nc.sync.dma_start(
                out=wv[:, 1, :, :],
                in_=wvT_d[:, 1024:2048].rearrange("p (k d) -> p k d", k=KT))
            wout = sbW.tile([128, 2, H], F16)
            nc.sync.dma_start(out=wout, in_=woutT_d.rearrange("(q p) e -> p q e", p=128))
            ident = sbW.tile([128, 128], F16)
            nc.sync.dma_start(out=ident, in_=ident_d[:, :])

            # persistent per-chunk q/k tiles, per-j-tile v tiles
            qTc = [[sbA.tile([128, 512], F32R, tag=f"qT{p}_{i}", name=f"qT{p}_{i}")
                    for i in range(IC)] for p in range(2)]
            kTc = [[sbA.tile([128, 512], F32R, tag=f"kT{p}_{i}", name=f"kT{p}_{i}")
                    for i in range(IC)] for p in range(2)]
            vtj = [sbA.tile([128, HEADS_PER_CORE, 65], F16, tag=f"vt{jt}",
                            name=f"vt{jt}") for jt in range(JT)]
            # ones column of v_aug, written once (on idle GPSIMD)
            for jt in range(JT):
                nc.gpsimd.memset(vtj[jt][:, :, 64:65], 1.0)
            o2T = [[sbA.tile([128, 512], F16, tag=f"o2T{p}_{ic}",
                             name=f"o2T{p}_{ic}")
                    for ic in range(IC)] for p in range(2)]

            # ---- chain emitters ------------------------------------------
            def q_mm(p, ic, ps, k):
                nc.tensor.matmul(ps, lhsT=wq[:, p, k, :],
                                 rhs=xt[ic][:, k, :],
                                 start=(k == 0), stop=(k == KT - 1))

            def k_mm(p, c, ps, k):
                nc.tensor.matmul(ps, lhsT=wk[:, p, k, :],
                                 rhs=xt[c][:, k, :],
                                 start=(k == 0), stop=(k == KT - 1))

            def q_fin(p, ic, ps):
                nc.vector.tensor_scalar_add(qTc[p][ic], ps, bqk_sb[:, p:p + 1])

            def k_fin(p, c, ps):
                nc.vector.tensor_scalar_add(kTc[p][c], ps, bqk_sb[:, 2 + p:3 + p])

            def chain_parts(kind, p, i, nparts=4):
                state = {}
                per = KT // nparts
                mm = q_mm if kind == "q" else k_mm
                fin = q_fin if kind == "q" else k_fin

                def part(j):
                    def f():
                        if j == 0:
                            state["ps"] = psCH.tile([128, 512], F32, tag="ch",
                                                    name=f"ps_{kind}")
                        for k in range(per * j, per * (j + 1)):
                            mm(p, i, state["ps"], k)
                        if j == nparts - 1:
                            fin(p, i, state["ps"])
                    return f
                return [part(j) for j in range(nparts)]

            def whole_chain(kind, p, i):
                for f in chain_parts(kind, p, i, nparts=1):
                    f()

            def v_chain(jt, vp):
                # v projection for ONE head pair: halves the v work the
                # first (PE-oversubscribed) chunk must absorb; pair-1's
                # chains ride the ACT-bound middle chunks' spare PE.
                c, jl = jt // 4, jt % 4
                ps = psCH.tile([128, 128], F32, tag="ch", name="ps_v")
                for k in range(KT):
                    nc.tensor.matmul(ps,
                                     lhsT=xt[c][:, k, 128 * jl:128 * jl + 128],
                                     rhs=wv[:, vp, k, :],
                                     start=(k == 0), stop=(k == KT - 1))
                nc.vector.tensor_add(
                    vtj[jt][:, 2 * vp:2 * vp + 2, 0:64],
                    ps.rearrange("p (h d) -> p h d", h=2),
                    bvrep.rearrange("p (h d) -> p h d",
                                    h=HEADS_PER_CORE)[:, 2 * vp:2 * vp + 2, :])

            def s_pair(p, ic, jt):
                c, jl = jt // 4, jt % 4
                st = psST.tile([128, 1024], F32, tag="st", name="st")
                nc.tensor.matmul(st[:, 0:512],
                                 lhsT=kTc[p][c][0:64, 128 * jl:128 * jl + 128],
                                 rhs=qTc[p][ic][0:64, :],
                                 start=True, stop=True)
                nc.tensor.matmul(st[:, 512:1024],
                                 lhsT=kTc[p][c][64:128, 128 * jl:128 * jl + 128],
                                 rhs=qTc[p][ic][64:128, :],
                                 start=True, stop=True)
                return st

            def pv_step(p, jt, pt, accs):
                for t in range(4):
                    for h in range(2):
                        o0 = 130 * (t % 2) + 65 * h
                        # start=True zeroes the whole 2KB PSUM bank
                        # (ZERO_REGION_SIZE): only the first matmul per acc
                        # tile may set it; later regions' first writes land
                        # on pending-zero bytes and initialize correctly.
                        nc.tensor.matmul(
                            accs[t // 2][:, o0:o0 + 65],
                            lhsT=pt[:, 512 * h + 128 * t:512 * h + 128 * t + 128],
                            rhs=vtj[jt][:, 2 * p + h, :],
                            start=(jt == 0 and t % 2 == 0 and h == 0),
                            stop=(jt == JT - 1 and t % 2 == 1 and h == 1),
                            skip_group_check=True)

            def attention_body(p, ic, fillers=None, st_in=None, nxt=None):
                # acc regions: accA holds q-subtiles 0,1; accB holds 2,3.
                # Region (t, h) = [:, 130*(t%2) + 65*h :][:65]; col 64 is z.
                accA = psACC.tile([128, 260], F32, tag="accA", name="accA")
                accB = psACC.tile([128, 260], F32, tag="accB", name="accB")
                accs = [accA, accB]

                st = st_in if st_in is not None else s_pair(p, ic, 0)
                st_out = None
                pts = [None, None]  # PV trails exp by one step
                for jt in range(JT):
                    if fillers:
                        for f in fillers.get(jt, ()):
                            f()
                    if jt + 1 < JT:
                        st_next = s_pair(p, ic, jt + 1)
                    elif nxt is not None:
                        st_next = st_out = s_pair(nxt[0], nxt[1], 0)
                    else:
                        st_next = None
                    pt = sbPT.tile([128, 1024], F16, tag="pt", name="pt")
                    nc.scalar.activation(pt, st, EXP, scale=float(HD) ** -0.5)
                    pts[jt % 2] = pt
                    if jt >= 1:
                        pv_step(p, jt - 1, pts[(jt - 1) % 2], accs)
                    st = st_next
                pv_step(p, JT - 1, pts[(JT - 1) % 2], accs)
                return (p, ic, accs), st_out

            def norm_mul(o2sb, accs, a, zrec):
                # One broadcast multiply per acc bank: o2sb[q, t, (h c)] =
                # acc regions * (1/z) with z stride-0-broadcast along c.
                out_v = o2sb[:, 2 * a:2 * a + 2, :].rearrange(
                    "p t (h c) -> p (t h) c", c=64)
                acc_v = accs[a].rearrange("p (r c) -> p r c", c=65)[:, :, 0:64]
                z_v = zrec[:, 4 * a:4 * a + 4].rearrange(
                    "p (r one) -> p r one", one=1)
                acc_b, z_b = bass.broadcast_tensor_aps(acc_v, z_v)
                nc.vector.tensor_mul(out_v, acc_b, z_b)

            def normalize_fin(pend):
                # o2sb[q, t, d2] = acc / z (per-partition scalars), then a
                # DMA-engine transpose flips to o2T[d2, t*128+q].
                p, ic, accs = pend
                zrec = sbZ.tile([128, 8], F32, tag="zrec", name="zrec")
                o2sb = sbZ.tile([128, 4, 128], F16, tag="o2sb", name="o2sb")
                # per-acc interleave: accA's slot releases after two ops so
                # the next chunk's first PV (WAR on that slot) starts sooner
                for a in range(2):
                    nc.vector.reciprocal(
                        zrec[:, 4 * a:4 * a + 4],
                        accs[a].rearrange("p (r c) -> p r c", c=65)[:, :, 64])
                    norm_mul(o2sb, accs, a, zrec)
                nc.sync.dma_start_transpose(
                    out=o2T[p][ic].rearrange("p (t q) -> p t q", t=4),
                    in_=o2sb.rearrange("p t q -> p (t q)"))

            def proj_unit(ic, itl, ec, ost_box, tail=False):
                t0 = 512 * ic + 128 * itl
                # in the tail the score banks are free: borrow them for a
                # deeper psum rotation so copies never pace the projections
                pool, tg = (psST, "st") if (tail and ec == 1) else (psCH, "ch")
                ps = pool.tile([128, 512], F32, tag=tg, name="ps_o")
                e0 = 512 * ec
                nc.tensor.matmul(ps,
                                 lhsT=o2T[0][ic][:, 128 * itl:128 * itl + 128],
                                 rhs=wout[:, 0, e0:e0 + 512],
                                 start=True, stop=False)
                nc.tensor.matmul(ps,
                                 lhsT=o2T[1][ic][:, 128 * itl:128 * itl + 128],
                                 rhs=wout[:, 1, e0:e0 + 512],
                                 start=False, stop=True)
                if ec == 0:
                    ost_box["t"] = sbOut.tile([128, 1024], F16, tag="ost",
                                              name="ost")
                ost = ost_box["t"]
                # One full-row store per itl halves the serialized HWDGE
                # holds.  In the tail the Scalar engine is idle: give it
                # half the PSUM->SBUF copies so DVE isn't the pacer.
                if tail and ec == 0:
                    nc.scalar.copy(ost[:, e0:e0 + 512], ps)
                else:
                    nc.vector.tensor_copy(ost[:, e0:e0 + 512], ps)
                if ec == 1:
                    nc.sync.dma_start(out=out_d[t0:t0 + 128, :], in_=ost)

            def proj_units(ic, itl, tail=False):
                box = {}
                return [
                    (lambda i, t, e, b: lambda: proj_unit(i, t, e, b, tail))(
                        ic, itl, ec, box)
                    for ec in range(2)
                ]

            # ---- emission order = engine execution order ------------------
            # PE p-state warmup: a dense stream of trivial matmuls keeps the
            # tensor engine continuously busy from ~0.3us (a DVE memset
            # seeds the operand, no DMA dependency) until the first real
            # chains (~10us, gated on the x chunk-0 DMA), so the ramp to
            # the 2.4GHz p-state is paid on throwaway work.
            wuseed = sbZ.tile([128, 2], F16, tag="wus", name="wus")
            nc.vector.memset(wuseed[:, :], 0.0)
            wups = psCH.tile([128, 16], F32, tag="ch", name="wu")
            for _ in range(880):
                nc.tensor.matmul(wups[0:2, 0:2], lhsT=wuseed[:, 0:2],
                                 rhs=wuseed[:, 0:2], start=True, stop=True,
                                 skip_group_check=True)
            # interleaved first chains: q and k advance together per k-tile
            # (subtile deps let each matmul start as its half-DMA lands)
            ps_q0 = psCH.tile([128, 512], F32, tag="ch", name="ps_q0")
            ps_k0 = psCH.tile([128, 512], F32, tag="ch", name="ps_k0")
            for k in range(KT):
                q_mm(0, 0, ps_q0, k)
                k_mm(0, 0, ps_k0, k)
            q_fin(0, 0, ps_q0)
            # split the first k bias-add: s(0) only reads keys 0-127, so it
            # can start after the first half lands
            nc.vector.tensor_scalar_add(kTc[0][0][:, 0:256], ps_k0[:, 0:256],
                                        bqk_sb[:, 2:3])
            nc.vector.tensor_scalar_add(kTc[0][0][:, 256:512], ps_k0[:, 256:512],
                                        bqk_sb[:, 2:3])

            segs = []
            f00 = {jt: [] for jt in range(JT)}
            for jt in range(JT):
                f00[max(0, jt - 1)].append((lambda j: lambda: v_chain(j, 0))(jt))
            for ci in range(1, IC):
                for j, f in enumerate(chain_parts("k", 0, ci, nparts=2)):
                    f00[4 * ci - 4 + 1 + j].append(f)
            for j, f in enumerate(chain_parts("q", 0, 1, nparts=2)):
                f00[13 + j].append(f)
            segs.append((0, 0, f00))

            # Filler load balancing: every middle seg stays under the
            # 16.6us ACT pace (6.3us filler budget).  q(1,ic) is only
            # needed by the END of seg (1,ic-1) (score handover), so those
            # chains ride late segs; pair-1 v-chains spread over segs 1-4.
            for ic in range(1, IC):
                fq = {}
                if ic == 1:
                    for j, f in enumerate(chain_parts("k", 1, 0)):
                        fq.setdefault(1 + j, []).append(f)
                    for j, f in enumerate(chain_parts("q", 1, 0)):
                        fq.setdefault(5 + j, []).append(f)
                    for j, vj in enumerate((0, 1)):
                        fq.setdefault(6 * j, []).append(
                            (lambda x: lambda: v_chain(x, 1))(vj))
                elif ic == 2:
                    for j, f in enumerate(chain_parts("k", 1, 1)):
                        fq.setdefault(1 + j, []).append(f)
                    for j, vj in enumerate((2, 3, 4, 5)):
                        fq.setdefault((0, 5, 6, 7)[j], []).append(
                            (lambda x: lambda: v_chain(x, 1))(vj))
                else:
                    for j, f in enumerate(chain_parts("k", 1, 2)):
                        fq.setdefault(1 + j, []).append(f)
                    for j, vj in enumerate((6, 7, 8, 9, 10, 11)):
                        fq.setdefault((0, 5, 6, 7, 8, 9)[j], []).append(
                            (lambda x: lambda: v_chain(x, 1))(vj))
                if ic < IC - 1:
                    for j, f in enumerate(chain_parts("q", 0, ic + 1)):
                        fq.setdefault(10 + j, []).append(f)
                segs.append((0, ic, fq))
            for ic in range(IC):
                fp = {}
                if ic == 0:
                    for j, vj in enumerate((12, 13, 14, 15)):
                        fp.setdefault((0, 1, 11, 12)[j], []).append(
                            (lambda x: lambda: v_chain(x, 1))(vj))
                    for j, f in enumerate(chain_parts("q", 1, 1)):
                        fp.setdefault(2 + j, []).append(f)
                    for j, f in enumerate(chain_parts("k", 1, 3)):
                        fp.setdefault(7 + j, []).append(f)
                else:
                    units = [u for itl in range(4) for u in proj_units(ic - 1, itl)]
                    for u, f in zip((4, 5, 6, 7, 8, 9, 11, 12), units):
                        fp.setdefault(u, []).append(f)
                    if ic < IC - 1:
                        for j, f in enumerate(chain_parts("q", 1, ic + 1)):
                            fp.setdefault(j, []).append(f)
                segs.append((1, ic, fp))

            st_hand = None
            pend = None
            for idx, (p, ic, fillers) in enumerate(segs):
                nxt = segs[idx + 1][:2] if idx + 1 < len(segs) else None
                pend, st_hand = attention_body(p, ic, fillers,
                                               st_in=st_hand, nxt=nxt)
                if idx + 1 < len(segs):
                    normalize_fin(pend)
            # tail: pipeline the last chunk per q-subtile so each subtile's
            # normalize -> transpose -> projection -> store chain starts as
            # soon as possible instead of after a monolithic transpose.
            p, ic, accs = pend
            zrec = sbZ.tile([128, 8], F32, tag="zrec", name="zrec")
            for a in range(2):
                nc.vector.reciprocal(
                    zrec[:, 4 * a:4 * a + 4],
                    accs[a].rearrange("p (r c) -> p r c", c=65)[:, :, 64])
            o2sb = sbZ.tile([128, 4, 128], F16, tag="o2sb", name="o2sb")
            for t in range(4):
                if t % 2 == 0:
                    # one mul per acc bank (covers two subtiles): shorter DVE
                    # chain, so the transposes start sooner
                    a = t // 2
                    out_v = o2sb[:, 2 * a:2 * a + 2, :].rearrange(
                        "p t (h c) -> p (t h) c", c=64)
                    acc_v = accs[a].rearrange("p (r c) -> p r c", c=65)[:, :, 0:64]
                    z_v = zrec[:, 4 * a:4 * a + 4].rearrange(
                        "p (r one) -> p r one", one=1)
                    acc_b, z_b = bass.broadcast_tensor_aps(acc_v, z_v)
                    nc.vector.tensor_mul(out_v, acc_b, z_b)
                # PE-side transpose: ~1us lower latency than the DMA path,
                # and the PE is idle in the tail anyway.  The transposes
                # borrow the acc banks (free after the norm reads) so they
                # don't rotate through the proj PSUM slots.
                pst = psACC.tile([128, 128], F16, tag="accA" if t % 2 == 0 else "accB",
                                 name="pst")
                nc.tensor.transpose(pst, o2sb[:, t, :], ident)
                # ACT has slack in the tail; keep the tiny proj-gating copy
                # off the DVE queue (behind the big ost copies)
                nc.scalar.copy(o2T[p][ic][:, 128 * t:128 * (t + 1)], pst)
                for f in proj_units(IC - 1, t, tail=True):
                    f()
    nc.compile()
    return nc


def _get_nc():
    if "nc" not in _CACHE:
        _CACHE["nc"] = _build()
    return _CACHE["nc"]


def make_in_maps(x, w_qkv, b_qkv, w_out):
    """Per-core input dicts for the SPMD kernel."""
    f32 = np.float32
    x = np.asarray(x, f32)
    w_qkv = np.asarray(w_qkv, f32)
    b_qkv = np.asarray(b_qkv, f32)
    w_out = np.asarray(w_out, f32)
    in_maps = []
    for c in range(N_CORES):
        b = c // 4
        g = c % 4
        r0, r1 = 64 * 4 * g, 64 * 4 * (g + 1)
        in_maps.append({
            "xT": np.ascontiguousarray(x[b].T).astype(np.float16),
            "wqT": np.ascontiguousarray(
                w_qkv[r0:r1, :].T.reshape(8, 128, 2, 128).transpose(1, 2, 0, 3)
                .reshape(128, 2048)).astype(np.float16),
            "wkT": np.ascontiguousarray(
                w_qkv[H + r0:H + r1, :].T.reshape(8, 128, 2, 128)
                .transpose(1, 2, 0, 3).reshape(128, 2048)).astype(np.float16),
            "wvT": np.ascontiguousarray(
                w_qkv[2 * H + r0:2 * H + r1, :].T.reshape(8, 128, 2, 128)
                .transpose(1, 2, 0, 3).reshape(128, 2048)).astype(np.float16),
            "woutT": np.ascontiguousarray(w_out[:, r0:r1].T).astype(np.float16),
            "bqk": np.concatenate([
                b_qkv[r0:r1].reshape(2, 128).T,
                b_qkv[H + r0:H + r1].reshape(2, 128).T], axis=1).copy(),
            "bvrep": np.tile(b_qkv[2 * H + r0:2 * H + r1][None, :], (128, 1)),
            "ident": np.eye(128, dtype=np.float16),
        })
    return in_maps


def assemble(results, b_out):
    """Sum per-core partials into the full [B, L, H] output."""
    out = np.empty((B, L, H), np.float32)
    for b in range(B):
        acc = results[4 * b]["out"].astype(np.float32)
        for c in range(4 * b + 1, 4 * b + 4):
            acc = acc + results[c]["out"]
        out[b] = acc + np.asarray(b_out, np.float32)[None, :]
    return out


def kernel(x, w_qkv, b_qkv, w_out, b_out):
    from concourse.bass_utils import run_bass_kernel_spmd

    nc = _get_nc()
    in_maps = make_in_maps(x, w_qkv, b_qkv, w_out)
    res = run_bass_kernel_spmd(nc, in_maps, core_ids=list(range(N_CORES)))
    return assemble(res.results, b_out)


if __name__ == "__main__":
    rng = np.random.default_rng(0)
    x = rng.standard_normal((B, L, H), dtype=np.float32)
    w_qkv = (rng.standard_normal((3 * H, H), dtype=np.float32) / np.sqrt(H)).astype(np.float32)
    b_qkv = (rng.standard_normal(3 * H).astype(np.float32) * 0.01)
    w_out = (rng.standard_normal((H, H), dtype=np.float32) / np.sqrt(H)).astype(np.float32)
    b_out = (rng.standard_normal(H).astype(np.float32) * 0.01)
    out = kernel(x, w_qkv, b_qkv, w_out, b_out)
    print("kernel output", out.shape, out.dtype)



# revision 2
# speedup vs baseline: 9.7952x; 9.7952x over previous
"""Trainium2 Bass kernel for a dense multi-head attention layer.

Reference computation (per batch b):
    qkv = x @ w_qkv.T + b_qkv                # [L, 3H]
    q, k, v per head (NH=16 heads, HD=64)
    attn = softmax((q @ k.T) * HD**-0.5)
    out  = (attn @ v) per head, concat, @ w_out.T + b_out

Sharding across 8 NeuronCores: core c handles batch b = c // 4 and the
4-head group g = c % 4 (heads 4g .. 4g+3, organized as 2 pairs of 2).
Each core computes its partial output projection [L, H]; the host sums
the 4 partials per batch and adds b_out.

Per-core on-device plan:
  - All 16-bit operand paths use fp16 (values are small, so fp16's extra
    mantissa beats bf16); q/k/scores stay fp32.  Matmul operand pairs are
    width-uniform (neuronxcc rejects 32-bit x 16-bit mixes).
  - x streams in as fp16 chunk tiles; weights fp16; a handful of big
    DMAs in dependency order (each DMA pays a ~625ns serialized HWDGE
    hold, so fewer + bigger wins the startup race).
  - qT/kT are [HD, L] per head (head pairs stacked on 128 partitions,
    fp32r); v is produced per 128-key tile in natural [keys, HD] fp16
    layout augmented with a ones column (so attn @ v_aug also yields the
    softmax denominator z).
  - Scores are computed transposed, ST[j, i] (keys on partitions), one
    128-key tile at a time, two heads row-tiled on the PE (K=64 each).
  - exp() on the Scalar engine with the 1/8 scale fused; max-subtraction
    is skipped (|scores/8| <= ~6, safely inside exp range).
  - PV runs TRANSPOSED: out[q, d] with queries on partitions, using pt
    128-query slices as the stationary operand and v_aug [128, 65] fp16
    moving -- 65-row matmuls at full rate, half the PE cycles of the
    [65, 512] orientation; z lands in column 64 with q on partitions so
    normalization is a native per-partition tensor_scalar on DVE.
    PV trails exp by one step so chunk-transition PSUM WAR waits hide
    behind the exp stream instead of stalling the in-order PE queue.
  - o2 [q, d] flips back to [d, q] for the output projection via a
    DMA-engine transpose (InstDmaTransposeAnt) -- zero PE/DVE cost.
  - Stores are full [128, 1024] fp16 rows (halves the serialized HWDGE
    holds); a tiny-matmul warmup stream holds the PE at its 2.4GHz
    p-state through the initial DMA window.
  - Fillers are globally load-balanced: the first chunk only carries its
    own head-pair's v-chains plus the pinned k-chains; pair-1 v-chains,
    later q/k projections and the output projection ride the exp slack of
    the ACT-bound middle chunks (every middle chunk stays under the
    16.6us exp pace).  Scores for step jt+1 are emitted ahead of exp(jt),
    carrying across chunk boundaries (st_in/nxt handover).
"""

import sys

sys.path.insert(0, "/opt/trn_rl_repo")

import numpy as np

H = 1024
NH = 16
HD = 64
L = 2048
B = 2
N_CORES = 8
HEADS_PER_CORE = 4
KT = H // 128  # 8 k-tiles over the hidden dim
IC = L // 512  # 4 i-chunks of 512 queries
JT = L // 128  # 16 j-tiles of 128 keys

_CACHE = {}


def _build():
    import concourse.bass as bass
    import concourse.mybir as mybir
    import concourse.tile as tile
    from concourse import bacc

    F32 = mybir.dt.float32
    F32R = mybir.dt.float32r
    F16 = mybir.dt.float16
    EXP = mybir.ActivationFunctionType.Exp

    nc = bacc.Bacc("TRN2", target_bir_lowering=False, debug=False,
                   num_devices=N_CORES)

    xT_d = nc.declare_dram_parameter("xT", [H, L], F16, isOutput=False)
    wqT_d = nc.declare_dram_parameter("wqT", [128, 2048], F16, isOutput=False)
    wkT_d = nc.declare_dram_parameter("wkT", [128, 2048], F16, isOutput=False)
    wvT_d = nc.declare_dram_parameter("wvT", [128, 2048], F16, isOutput=False)
    woutT_d = nc.declare_dram_parameter("woutT", [256, H], F16, isOutput=False)
    bqk_d = nc.declare_dram_parameter("bqk", [128, 4], F32, isOutput=False)
    bvrep_d = nc.declare_dram_parameter("bvrep", [128, 256], F32, isOutput=False)
    ident_d = nc.declare_dram_parameter("ident", [128, 128], F16, isOutput=False)
    out_d = nc.declare_dram_parameter("out", [L, H], F16, isOutput=True)

    with tile.TileContext(nc) as tc, nc.allow_low_precision(
            reason="fp16 operand tiles; all reductions accumulate in fp32 "
                   "PSUM"):
        with tc.tile_pool(name="sbW", bufs=1) as sbW, \
             tc.tile_pool(name="sbA", bufs=1) as sbA, \
             tc.tile_pool(name="sbPT", bufs=4) as sbPT, \
             tc.tile_pool(name="sbZ", bufs=2) as sbZ, \
             tc.tile_pool(name="sbOut", bufs=4) as sbOut, \
             tc.tile_pool(name="psST", bufs=2, space="PSUM") as psST, \
             tc.tile_pool(name="psACC", bufs=1, space="PSUM") as psACC, \
             tc.tile_pool(name="psCH", bufs=2, space="PSUM") as psCH:

            # ---- resident loads: few big DMAs, dependency order ----------
            # Transfer order IS the critical path: wq then x chunk 0 then wk
            # gets the q-chain going at ~6us.
            xT_r = xT_d.rearrange("(k p) n -> p k n", p=128)
            # First-use tensors stream in halves (k-tiles 0-3, then 4-7) so
            # the q/k chains can start ~2us earlier.
            # wq/wk are pair-major [p, pair, k, d]: the first chains need
            # only pair-0's half, and the partition-major DRAM layout keeps
            # every DMA run at 2KB (no sub-512B descriptor penalty).
            wq = sbW.tile([128, 2, KT, 128], F16, tag="wq", name="wq")
            wk = sbW.tile([128, 2, KT, 128], F16, tag="wk", name="wk")
            x0 = sbW.tile([128, KT, 512], F16, tag="x0", name="x0")
            nc.sync.dma_start(out=x0[:, 0:4, :], in_=xT_r[:, 0:4, 0:512])
            nc.sync.dma_start(
                out=wq[:, 0, :, :],
                in_=wqT_d[:, 0:1024].rearrange("p (k d) -> p k d", k=KT))
            nc.sync.dma_start(
                out=wk[:, 0, :, :],
                in_=wkT_d[:, 0:1024].rearrange("p (k d) -> p k d", k=KT))
            nc.sync.dma_start(out=x0[:, 4:8, :], in_=xT_r[:, 4:8, 0:512])
            bqk_sb = sbW.tile([128, 4], F32)
            nc.sync.dma_start(out=bqk_sb, in_=bqk_d[:, :])
            wv = sbW.tile([128, 2, KT, 128], F16, tag="wv", name="wv")
            nc.sync.dma_start(
                out=wv[:, 0, :, :],
                in_=wvT_d[:, 0:1024].rearrange("p (k d) -> p k d", k=KT))
            xt = [x0]
            x1 = sbW.tile([128, KT, 512], F16, tag="x1", name="x1")
            nc.sync.dma_start(out=x1, in_=xT_r[:, :, 512:1024])
            xt.append(x1)
            # x2/x3 feed seg0 fillers at ~18us; the pair-1 weights are not
            # needed until ~35us -- keep x ahead of them in the DMA queue
            bvrep = sbW.tile([128, 256], F32)
            nc.sync.dma_start(out=bvrep, in_=bvrep_d[:, :])
            for c in range(2, IC):
                xc = sbW.tile([128, KT, 512], F16, tag=f"x{c}", name=f"x{c}")
                nc.sync.dma_start(out=xc, in_=xT_r[:, :, 512 * c:512 * (c + 1)])
                xt.append(xc)
            nc.sync.dma_start(
                out=wq[:, 1, :, :],
                in_=wqT_d[:, 1024:2048].rearrange("p (k d) -> p k d", k=KT))
            nc.sync.dma_start(
                out=wk[:, 1, :, :],
                in_=wkT_d[:, 1024:2048].rearrange("p (k d) -> p k d", k=KT))
            nc.sync.dma_start(
                out=wv[:, 1, :, :],
                in_=wvT_d[:, 1024:2048].rearrange("p (k d) -> p k d", k=KT))
            wout = sbW.tile([128, 2, H], F16)
            nc.sync.dma_start(out=wout, in_=woutT_d.rearrange("(q p) e -> p q e", p=128))
            ident = sbW.tile([128, 128], F16)
            nc.sync.dma_start(out=ident, in_=ident_d[:, :])

            # persistent per-chunk q/k tiles, per-j-tile v tiles
            qTc = [[sbA.tile([128, 512], F32R, tag=f"qT{p}_{i}", name=f"qT{p}_{i}")
                    for i in range(IC)] for p in range(2)]
            kTc = [[sbA.tile([128, 512], F32R, tag=f"kT{p}_{i}", name=f"kT{p}_{i}")
                    for i in range(IC)] for p in range(2)]
            vtj = [sbA.tile([128, HEADS_PER_CORE, 65], F16, tag=f"vt{jt}",
                            name=f"vt{jt}") for jt in range(JT)]
            # ones column of v_aug, written once (on idle GPSIMD)
            for jt in range(JT):
                nc.gpsimd.memset(vtj[jt][:, :, 64:65], 1.0)
            o2T = [[sbA.tile([128, 512], F16, tag=f"o2T{p}_{ic}",
                             name=f"o2T{p}_{ic}")
                    for ic in range(IC)] for p in range(2)]

            # ---- chain emitters ------------------------------------------
            def q_mm(p, ic, ps, k):
                nc.tensor.matmul(ps, lhsT=wq[:, p, k, :],
                                 rhs=xt[ic][:, k, :],
                                 start=(k == 0), stop=(k == KT - 1))

            def k_mm(p, c, ps, k):
                nc.tensor.matmul(ps, lhsT=wk[:, p, k, :],
                                 rhs=xt[c][:, k, :],
                                 start=(k == 0), stop=(k == KT - 1))

            def q_fin(p, ic, ps):
                nc.vector.tensor_scalar_add(qTc[p][ic], ps, bqk_sb[:, p:p + 1])

            def k_fin(p, c, ps):
                nc.vector.tensor_scalar_add(kTc[p][c], ps, bqk_sb[:, 2 + p:3 + p])

            def chain_parts(kind, p, i, nparts=4):
                state = {}
                per = KT // nparts
                mm = q_mm if kind == "q" else k_mm
                fin = q_fin if kind == "q" else k_fin

                def part(j):
                    def f():
                        if j == 0:
                            state["ps"] = psCH.tile([128, 512], F32, tag="ch",
                                                    name=f"ps_{kind}")
                        for k in range(per * j, per * (j + 1)):
                            mm(p, i, state["ps"], k)
                        if j == nparts - 1:
                            fin(p, i, state["ps"])
                    return f
                return [part(j) for j in range(nparts)]

            def whole_chain(kind, p, i):
                for f in chain_parts(kind, p, i, nparts=1):
                    f()

            def v_chain(jt, vp):
                # v projection for ONE head pair: halves the v work the
                # first (PE-oversubscribed) chunk must absorb; pair-1's
                # chains ride the ACT-bound middle chunks' spare PE.
                c, jl = jt // 4, jt % 4
                ps = psCH.tile([128, 128], F32, tag="ch", name="ps_v")
                for k in range(KT):
                    nc.tensor.matmul(ps,
                                     lhsT=xt[c][:, k, 128 * jl:128 * jl + 128],
                                     rhs=wv[:, vp, k, :],
                                     start=(k == 0), stop=(k == KT - 1))
                nc.vector.tensor_add(
                    vtj[jt][:, 2 * vp:2 * vp + 2, 0:64],
                    ps.rearrange("p (h d) -> p h d", h=2),
                    bvrep.rearrange("p (h d) -> p h d",
                                    h=HEADS_PER_CORE)[:, 2 * vp:2 * vp + 2, :])

            def s_pair(p, ic, jt):
                c, jl = jt // 4, jt % 4
                st = psST.tile([128, 1024], F32, tag="st", name="st")
                nc.tensor.matmul(st[:, 0:512],
                                 lhsT=kTc[p][c][0:64, 128 * jl:128 * jl + 128],
                                 rhs=qTc[p][ic][0:64, :],
                                 start=True, stop=True)
                nc.tensor.matmul(st[:, 512:1024],
                                 lhsT=kTc[p][c][64:128, 128 * jl:128 * jl + 128],
                                 rhs=qTc[p][ic][64:128, :],
                                 start=True, stop=True)
                return st

            def pv_step(p, jt, pt, accs):
                for t in range(4):
                    for h in range(2):
                        o0 = 130 * (t % 2) + 65 * h
                        # start=True zeroes the whole 2KB PSUM bank
                        # (ZERO_REGION_SIZE): only the first matmul per acc
                        # tile may set it; later regions' first writes land
                        # on pending-zero bytes and initialize correctly.
                        nc.tensor.matmul(
                            accs[t // 2][:, o0:o0 + 65],
                            lhsT=pt[:, 512 * h + 128 * t:512 * h + 128 * t + 128],
                            rhs=vtj[jt][:, 2 * p + h, :],
                            start=(jt == 0 and t % 2 == 0 and h == 0),
                            stop=(jt == JT - 1 and t % 2 == 1 and h == 1),
                            skip_group_check=True)

            def attention_body(p, ic, fillers=None, st_in=None, nxt=None):
                # acc regions: accA holds q-subtiles 0,1; accB holds 2,3.
                # Region (t, h) = [:, 130*(t%2) + 65*h :][:65]; col 64 is z.
                accA = psACC.tile([128, 260], F32, tag="accA", name="accA")
                accB = psACC.tile([128, 260], F32, tag="accB", name="accB")
                accs = [accA, accB]

                st = st_in if st_in is not None else s_pair(p, ic, 0)
                st_out = None
                pts = [None, None]  # PV trails exp by one step
                for jt in range(JT):
                    if fillers:
                        for f in fillers.get(jt, ()):
                            f()
                    if jt + 1 < JT:
                        st_next = s_pair(p, ic, jt + 1)
                    elif nxt is not None:
                        st_next = st_out = s_pair(nxt[0], nxt[1], 0)
                    else:
                        st_next = None
                    pt = sbPT.tile([128, 1024], F16, tag="pt", name="pt")
                    nc.scalar.activation(pt, st, EXP, scale=float(HD) ** -0.5)
                    pts[jt % 2] = pt
                    if jt >= 1:
                        pv_step(p, jt - 1, pts[(jt - 1) % 2], accs)
                    st = st_next
                pv_step(p, JT - 1, pts[(JT - 1) % 2], accs)
                return (p, ic, accs), st_out

            def norm_mul(o2sb, accs, a, zrec):
                # One broadcast multiply per acc bank: o2sb[q, t, (h c)] =
                # acc regions * (1/z) with z stride-0-broadcast along c.
                out_v = o2sb[:, 2 * a:2 * a + 2, :].rearrange(
                    "p t (h c) -> p (t h) c", c=64)
                acc_v = accs[a].rearrange("p (r c) -> p r c", c=65)[:, :, 0:64]
                z_v = zrec[:, 4 * a:4 * a + 4].rearrange(
                    "p (r one) -> p r one", one=1)
                acc_b, z_b = bass.broadcast_tensor_aps(acc_v, z_v)
                nc.vector.tensor_mul(out_v, acc_b, z_b)

            def normalize_fin(pend):
                # o2sb[q, t, d2] = acc / z (per-partition scalars), then a
                # DMA-engine transpose flips to o2T[d2, t*128+q].
                p, ic, accs = pend
                zrec = sbZ.tile([128, 8], F32, tag="zrec", name="zrec")
                o2sb = sbZ.tile([128, 4, 128], F16, tag="o2sb", name="o2sb")
                # per-acc interleave: accA's slot releases after two ops so
                # the next chunk's first PV (WAR on that slot) starts sooner
                for a in range(2):
                    nc.vector.reciprocal(
                        zrec[:, 4 * a:4 * a + 4],
                        accs[a].rearrange("p (r c) -> p r c", c=65)[:, :, 64])
                    norm_mul(o2sb, accs, a, zrec)
                nc.sync.dma_start_transpose(
                    out=o2T[p][ic].rearrange("p (t q) -> p t q", t=4),
                    in_=o2sb.rearrange("p t q -> p (t q)"))

            def proj_unit(ic, itl, ec, ost_box, tail=False):
                t0 = 512 * ic + 128 * itl
                # in the tail the score banks are free: borrow them for a
                # deeper psum rotation so copies never pace the projections
                pool, tg = (psST, "st") if (tail and ec == 1) else (psCH, "ch")
                ps = pool.tile([128, 512], F32, tag=tg, name="ps_o")
                e0 = 512 * ec
                nc.tensor.matmul(ps,
                                 lhsT=o2T[0][ic][:, 128 * itl:128 * itl + 128],
                                 rhs=wout[:, 0, e0:e0 + 512],
                                 start=True, stop=False)
                nc.tensor.matmul(ps,
                                 lhsT=o2T[1][ic][:, 128 * itl:128 * itl + 128],
                                 rhs=wout[:, 1, e0:e0 + 512],
                                 start=False, stop=True)
                if ec == 0:
                    ost_box["t"] = sbOut.tile([128, 1024], F16, tag="ost",
                                              name="ost")
                ost = ost_box["t"]
                # One full-row store per itl halves the serialized HWDGE
                # holds.  In the tail the Scalar engine is idle: give it
                # half the PSUM->SBUF copies so DVE isn't the pacer.
                if tail and ec == 0:
                    nc.scalar.copy(ost[:, e0:e0 + 512], ps)
                else:
                    nc.vector.tensor_copy(ost[:, e0:e0 + 512], ps)
                if tail:
                    # Half-row stores issued as each copy lands; odd itls ride
                    # the GPSIMD SWDGE queue so the two DGE front-ends drain
                    # the tail stores in parallel instead of serializing on
                    # the HWDGE's per-DMA hold.
                    dma = nc.gpsimd.dma_start if itl >= 2 else nc.sync.dma_start
                    dma(out=out_d[t0:t0 + 128, e0:e0 + 512],
                        in_=ost[:, e0:e0 + 512])
                elif ec == 1:
                    nc.sync.dma_start(out=out_d[t0:t0 + 128, :], in_=ost)

            def proj_units(ic, itl, tail=False):
                box = {}
                return [
                    (lambda i, t, e, b: lambda: proj_unit(i, t, e, b, tail))(
                        ic, itl, ec, box)
                    for ec in range(2)
                ]

            # ---- emission order = engine execution order ------------------
            # PE p-state warmup: a dense stream of trivial matmuls keeps the
            # tensor engine continuously busy from ~0.3us (a DVE memset
            # seeds the operand, no DMA dependency) until the first real
            # chains (~10us, gated on the x chunk-0 DMA), so the ramp to
            # the 2.4GHz p-state is paid on throwaway work.
            wuseed = sbZ.tile([128, 2], F16, tag="wus", name="wus")
            nc.vector.memset(wuseed[:, :], 0.0)
            wups = psCH.tile([128, 16], F32, tag="ch", name="wu")
            for _ in range(880):
                nc.tensor.matmul(wups[0:2, 0:2], lhsT=wuseed[:, 0:2],
                                 rhs=wuseed[:, 0:2], start=True, stop=True,
                                 skip_group_check=True)
            # interleaved first chains: q and k advance together per k-tile
            # (subtile deps let each matmul start as its half-DMA lands)
            ps_q0 = psCH.tile([128, 512], F32, tag="ch", name="ps_q0")
            ps_k0 = psCH.tile([128, 512], F32, tag="ch", name="ps_k0")
            for k in range(KT):
                q_mm(0, 0, ps_q0, k)
                k_mm(0, 0, ps_k0, k)
            q_fin(0, 0, ps_q0)
            # split the first k bias-add: s(0) only reads keys 0-127, so it
            # can start after the first half lands
            nc.vector.tensor_scalar_add(kTc[0][0][:, 0:256], ps_k0[:, 0:256],
                                        bqk_sb[:, 2:3])
            nc.vector.tensor_scalar_add(kTc[0][0][:, 256:512], ps_k0[:, 256:512],
                                        bqk_sb[:, 2:3])

            segs = []
            f00 = {jt: [] for jt in range(JT)}
            for jt in range(JT):
                f00[max(0, jt - 1)].append((lambda j: lambda: v_chain(j, 0))(jt))
            for ci in range(1, IC):
                for j, f in enumerate(chain_parts("k", 0, ci, nparts=2)):
                    f00[4 * ci - 4 + 1 + j].append(f)
            for j, f in enumerate(chain_parts("q", 0, 1, nparts=2)):
                f00[13 + j].append(f)
            segs.append((0, 0, f00))

            # Filler load balancing: every middle seg stays under the
            # 16.6us ACT pace (6.3us filler budget).  q(1,ic) is only
            # needed by the END of seg (1,ic-1) (score handover), so those
            # chains ride late segs; pair-1 v-chains spread over segs 1-4.
            for ic in range(1, IC):
                fq = {}
                if ic == 1:
                    for j, f in enumerate(chain_parts("k", 1, 0)):
                        fq.setdefault(1 + j, []).append(f)
                    for j, f in enumerate(chain_parts("q", 1, 0)):
                        fq.setdefault(5 + j, []).append(f)
                    for j, vj in enumerate((0, 1)):
                        fq.setdefault(6 * j, []).append(
                            (lambda x: lambda: v_chain(x, 1))(vj))
                elif ic == 2:
                    for j, f in enumerate(chain_parts("k", 1, 1)):
                        fq.setdefault(1 + j, []).append(f)
                    for j, vj in enumerate((2, 3, 4, 5)):
                        fq.setdefault((0, 5, 6, 7)[j], []).append(
                            (lambda x: lambda: v_chain(x, 1))(vj))
                else:
                    for j, f in enumerate(chain_parts("k", 1, 2)):
                        fq.setdefault(1 + j, []).append(f)
                    for j, vj in enumerate((6, 7, 8, 9, 10, 11)):
                        fq.setdefault((0, 5, 6, 7, 8, 9)[j], []).append(
                            (lambda x: lambda: v_chain(x, 1))(vj))
                if ic < IC - 1:
                    for j, f in enumerate(chain_parts("q", 0, ic + 1)):
                        fq.setdefault(10 + j, []).append(f)
                segs.append((0, ic, fq))
            for ic in range(IC):
                fp = {}
                if ic == 0:
                    for j, vj in enumerate((12, 13, 14, 15)):
                        fp.setdefault((0, 1, 11, 12)[j], []).append(
                            (lambda x: lambda: v_chain(x, 1))(vj))
                    for j, f in enumerate(chain_parts("q", 1, 1)):
                        fp.setdefault(2 + j, []).append(f)
                    for j, f in enumerate(chain_parts("k", 1, 3)):
                        fp.setdefault(7 + j, []).append(f)
                else:
                    units = [u for itl in range(4) for u in proj_units(ic - 1, itl)]
                    for u, f in zip((4, 5, 6, 7, 8, 9, 11, 12), units):
                        fp.setdefault(u, []).append(f)
                    if ic < IC - 1:
                        for j, f in enumerate(chain_parts("q", 1, ic + 1)):
                            fp.setdefault(j, []).append(f)
                segs.append((1, ic, fp))

            st_hand = None
            pend = None
            for idx, (p, ic, fillers) in enumerate(segs):
                nxt = segs[idx + 1][:2] if idx + 1 < len(segs) else None
                pend, st_hand = attention_body(p, ic, fillers,
                                               st_in=st_hand, nxt=nxt)
                if idx + 1 < len(segs):
                    normalize_fin(pend)
            # tail: pipeline the last chunk per q-subtile so each subtile's
            # normalize -> transpose -> projection -> store chain starts as
            # soon as possible instead of after a monolithic transpose.
            p, ic, accs = pend
            zrec = sbZ.tile([128, 8], F32, tag="zrec", name="zrec")
            for a in range(2):
                nc.vector.reciprocal(
                    zrec[:, 4 * a:4 * a + 4],
                    accs[a].rearrange("p (r c) -> p r c", c=65)[:, :, 64])
            o2sb = sbZ.tile([128, 4, 128], F16, tag="o2sb", name="o2sb")
            for t in range(4):
                if t % 2 == 0:
                    # one mul per acc bank (covers two subtiles): shorter DVE
                    # chain, so the transposes start sooner
                    a = t // 2
                    out_v = o2sb[:, 2 * a:2 * a + 2, :].rearrange(
                        "p t (h c) -> p (t h) c", c=64)
                    acc_v = accs[a].rearrange("p (r c) -> p r c", c=65)[:, :, 0:64]
                    z_v = zrec[:, 4 * a:4 * a + 4].rearrange(
                        "p (r one) -> p r one", one=1)
                    acc_b, z_b = bass.broadcast_tensor_aps(acc_v, z_v)
                    nc.vector.tensor_mul(out_v, acc_b, z_b)
                # PE-side transpose: ~1us lower latency than the DMA path,
                # and the PE is idle in the tail anyway.  The transposes
                # borrow the acc banks (free after the norm reads) so they
                # don't rotate through the proj PSUM slots.
                pst = psACC.tile([128, 128], F16, tag="accA" if t % 2 == 0 else "accB",
                                 name="pst")
                nc.tensor.transpose(pst, o2sb[:, t, :], ident)
                # ACT has slack in the tail; keep the tiny proj-gating copy
                # off the DVE queue (behind the big ost copies)
                nc.scalar.copy(o2T[p][ic][:, 128 * t:128 * (t + 1)], pst)
                for f in proj_units(IC - 1, t, tail=True):
                    f()
    nc.compile()
    return nc


def _get_nc():
    if "nc" not in _CACHE:
        _CACHE["nc"] = _build()
    return _CACHE["nc"]


def make_in_maps(x, w_qkv, b_qkv, w_out):
    """Per-core input dicts for the SPMD kernel."""
    f32 = np.float32
    x = np.asarray(x, f32)
    w_qkv = np.asarray(w_qkv, f32)
    b_qkv = np.asarray(b_qkv, f32)
    w_out = np.asarray(w_out, f32)
    in_maps = []
    for c in range(N_CORES):
        b = c // 4
        g = c % 4
        r0, r1 = 64 * 4 * g, 64 * 4 * (g + 1)
        in_maps.append({
            "xT": np.ascontiguousarray(x[b].T).astype(np.float16),
            "wqT": np.ascontiguousarray(
                w_qkv[r0:r1, :].T.reshape(8, 128, 2, 128).transpose(1, 2, 0, 3)
                .reshape(128, 2048)).astype(np.float16),
            "wkT": np.ascontiguousarray(
                w_qkv[H + r0:H + r1, :].T.reshape(8, 128, 2, 128)
                .transpose(1, 2, 0, 3).reshape(128, 2048)).astype(np.float16),
            "wvT": np.ascontiguousarray(
                w_qkv[2 * H + r0:2 * H + r1, :].T.reshape(8, 128, 2, 128)
                .transpose(1, 2, 0, 3).reshape(128, 2048)).astype(np.float16),
            "woutT": np.ascontiguousarray(w_out[:, r0:r1].T).astype(np.float16),
            "bqk": np.concatenate([
                b_qkv[r0:r1].reshape(2, 128).T,
                b_qkv[H + r0:H + r1].reshape(2, 128).T], axis=1).copy(),
            "bvrep": np.tile(b_qkv[2 * H + r0:2 * H + r1][None, :], (128, 1)),
            "ident": np.eye(128, dtype=np.float16),
        })
    return in_maps


def assemble(results, b_out):
    """Sum per-core partials into the full [B, L, H] output."""
    out = np.empty((B, L, H), np.float32)
    for b in range(B):
        acc = results[4 * b]["out"].astype(np.float32)
        for c in range(4 * b + 1, 4 * b + 4):
            acc = acc + results[c]["out"]
        out[b] = acc + np.asarray(b_out, np.float32)[None, :]
    return out


def kernel(x, w_qkv, b_qkv, w_out, b_out):
    from concourse.bass_utils import run_bass_kernel_spmd

    nc = _get_nc()
    in_maps = make_in_maps(x, w_qkv, b_qkv, w_out)
    res = run_bass_kernel_spmd(nc, in_maps, core_ids=list(range(N_CORES)))
    return assemble(res.results, b_out)


if __name__ == "__main__":
    rng = np.random.default_rng(0)
    x = rng.standard_normal((B, L, H), dtype=np.float32)
    w_qkv = (rng.standard_normal((3 * H, H), dtype=np.float32) / np.sqrt(H)).astype(np.float32)
    b_qkv = (rng.standard_normal(3 * H).astype(np.float32) * 0.01)
    w_out = (rng.standard_normal((H, H), dtype=np.float32) / np.sqrt(H)).astype(np.float32)
    b_out = (rng.standard_normal(H).astype(np.float32) * 0.01)
    out = kernel(x, w_qkv, b_qkv, w_out, b_out)
    print("kernel output", out.shape, out.dtype)



# revision 7
# speedup vs baseline: 9.9413x; 1.0149x over previous
"""Trainium2 Bass kernel for a dense multi-head attention layer.

Reference computation (per batch b):
    qkv = x @ w_qkv.T + b_qkv                # [L, 3H]
    q, k, v per head (NH=16 heads, HD=64)
    attn = softmax((q @ k.T) * HD**-0.5)
    out  = (attn @ v) per head, concat, @ w_out.T + b_out

Sharding across 8 NeuronCores: core c handles batch b = c // 4 and the
4-head group g = c % 4 (heads 4g .. 4g+3, organized as 2 pairs of 2).
Each core computes its partial output projection [L, H]; the host sums
the 4 partials per batch and adds b_out.

Per-core on-device plan:
  - All 16-bit operand paths use fp16 (values are small, so fp16's extra
    mantissa beats bf16); q/k/scores stay fp32.  Matmul operand pairs are
    width-uniform (neuronxcc rejects 32-bit x 16-bit mixes).
  - x streams in as fp16 chunk tiles; weights fp16; a handful of big
    DMAs in dependency order (each DMA pays a ~625ns serialized HWDGE
    hold, so fewer + bigger wins the startup race).
  - qT/kT are [HD, L] per head (head pairs stacked on 128 partitions,
    fp32r); v is produced per 128-key tile in natural [keys, HD] fp16
    layout augmented with a ones column (so attn @ v_aug also yields the
    softmax denominator z).
  - Scores are computed transposed, ST[j, i] (keys on partitions), one
    128-key tile at a time, two heads row-tiled on the PE (K=64 each).
  - exp() on the Scalar engine with the 1/8 scale fused; max-subtraction
    is skipped (|scores/8| <= ~6, safely inside exp range).
  - PV runs TRANSPOSED: out[q, d] with queries on partitions, using pt
    128-query slices as the stationary operand and v_aug [128, 65] fp16
    moving -- 65-row matmuls at full rate, half the PE cycles of the
    [65, 512] orientation; z lands in column 64 with q on partitions so
    normalization is a native per-partition tensor_scalar on DVE.
    PV trails exp by one step so chunk-transition PSUM WAR waits hide
    behind the exp stream instead of stalling the in-order PE queue.
  - o2 [q, d] flips back to [d, q] for the output projection via a
    DMA-engine transpose (InstDmaTransposeAnt) -- zero PE/DVE cost.
  - Stores are full [128, 1024] fp16 rows (halves the serialized HWDGE
    holds); a tiny-matmul warmup stream holds the PE at its 2.4GHz
    p-state through the initial DMA window.
  - Fillers are globally load-balanced: the first chunk only carries its
    own head-pair's v-chains plus the pinned k-chains; pair-1 v-chains,
    later q/k projections and the output projection ride the exp slack of
    the ACT-bound middle chunks (every middle chunk stays under the
    16.6us exp pace).  Scores for step jt+1 are emitted ahead of exp(jt),
    carrying across chunk boundaries (st_in/nxt handover).
"""

import sys

sys.path.insert(0, "/opt/trn_rl_repo")

import numpy as np

H = 1024
NH = 16
HD = 64
L = 2048
B = 2
N_CORES = 8
HEADS_PER_CORE = 4
KT = H // 128  # 8 k-tiles over the hidden dim
IC = L // 512  # 4 i-chunks of 512 queries
JT = L // 128  # 16 j-tiles of 128 keys

_CACHE = {}


def _build():
    import concourse.bass as bass
    import concourse.mybir as mybir
    import concourse.tile as tile
    from concourse import bacc

    F32 = mybir.dt.float32
    F32R = mybir.dt.float32r
    F16 = mybir.dt.float16
    EXP = mybir.ActivationFunctionType.Exp

    nc = bacc.Bacc("TRN2", target_bir_lowering=False, debug=False,
                   num_devices=N_CORES)

    xT_d = nc.declare_dram_parameter("xT", [H, L], F16, isOutput=False)
    wqT_d = nc.declare_dram_parameter("wqT", [128, 2048], F16, isOutput=False)
    wkT_d = nc.declare_dram_parameter("wkT", [128, 2048], F16, isOutput=False)
    wvT_d = nc.declare_dram_parameter("wvT", [128, 2048], F16, isOutput=False)
    woutT_d = nc.declare_dram_parameter("woutT", [256, H], F16, isOutput=False)
    bqk_d = nc.declare_dram_parameter("bqk", [128, 4], F32, isOutput=False)
    bvrep_d = nc.declare_dram_parameter("bvrep", [128, 256], F32, isOutput=False)
    ident_d = nc.declare_dram_parameter("ident", [128, 128], F16, isOutput=False)
    out_d = nc.declare_dram_parameter("out", [L, H], F16, isOutput=True)

    with tile.TileContext(nc) as tc, nc.allow_low_precision(
            reason="fp16 operand tiles; all reductions accumulate in fp32 "
                   "PSUM"):
        with tc.tile_pool(name="sbW", bufs=1) as sbW, \
             tc.tile_pool(name="sbA", bufs=1) as sbA, \
             tc.tile_pool(name="sbPT", bufs=4) as sbPT, \
             tc.tile_pool(name="sbZ", bufs=2) as sbZ, \
             tc.tile_pool(name="sbOut", bufs=4) as sbOut, \
             tc.tile_pool(name="psST", bufs=2, space="PSUM") as psST, \
             tc.tile_pool(name="psACC", bufs=1, space="PSUM") as psACC, \
             tc.tile_pool(name="psCH", bufs=2, space="PSUM") as psCH:

            # ---- resident loads: few big DMAs, dependency order ----------
            # Transfer order IS the critical path: wq then x chunk 0 then wk
            # gets the q-chain going at ~6us.
            xT_r = xT_d.rearrange("(k p) n -> p k n", p=128)
            # First-use tensors stream in halves (k-tiles 0-3, then 4-7) so
            # the q/k chains can start ~2us earlier.
            # wq/wk are pair-major [p, pair, k, d]: the first chains need
            # only pair-0's half, and the partition-major DRAM layout keeps
            # every DMA run at 2KB (no sub-512B descriptor penalty).
            wq = sbW.tile([128, 2, KT, 128], F16, tag="wq", name="wq")
            wk = sbW.tile([128, 2, KT, 128], F16, tag="wk", name="wk")
            x0 = sbW.tile([128, KT, 512], F16, tag="x0", name="x0")
            nc.sync.dma_start(out=x0[:, 0:4, :], in_=xT_r[:, 0:4, 0:512])
            nc.sync.dma_start(
                out=wq[:, 0, :, :],
                in_=wqT_d[:, 0:1024].rearrange("p (k d) -> p k d", k=KT))
            nc.sync.dma_start(
                out=wk[:, 0, :, :],
                in_=wkT_d[:, 0:1024].rearrange("p (k d) -> p k d", k=KT))
            nc.sync.dma_start(out=x0[:, 4:8, :], in_=xT_r[:, 4:8, 0:512])
            bqk_sb = sbW.tile([128, 4], F32)
            nc.sync.dma_start(out=bqk_sb, in_=bqk_d[:, :])
            wv = sbW.tile([128, 2, KT, 128], F16, tag="wv", name="wv")
            nc.sync.dma_start(
                out=wv[:, 0, :, :],
                in_=wvT_d[:, 0:1024].rearrange("p (k d) -> p k d", k=KT))
            xt = [x0]
            x1 = sbW.tile([128, KT, 512], F16, tag="x1", name="x1")
            nc.sync.dma_start(out=x1, in_=xT_r[:, :, 512:1024])
            xt.append(x1)
            # x2/x3 feed seg0 fillers at ~18us; the pair-1 weights are not
            # needed until ~35us -- keep x ahead of them in the DMA queue
            bvrep = sbW.tile([128, 256], F32)
            nc.sync.dma_start(out=bvrep, in_=bvrep_d[:, :])
            for c in range(2, IC):
                xc = sbW.tile([128, KT, 512], F16, tag=f"x{c}", name=f"x{c}")
                nc.sync.dma_start(out=xc, in_=xT_r[:, :, 512 * c:512 * (c + 1)])
                xt.append(xc)
            nc.sync.dma_start(
                out=wq[:, 1, :, :],
                in_=wqT_d[:, 1024:2048].rearrange("p (k d) -> p k d", k=KT))
            nc.sync.dma_start(
                out=wk[:, 1, :, :],
                in_=wkT_d[:, 1024:2048].rearrange("p (k d) -> p k d", k=KT))
            nc.sync.dma_start(
                out=wv[:, 1, :, :],
                in_=wvT_d[:, 1024:2048].rearrange("p (k d) -> p k d", k=KT))
            wout = sbW.tile([128, 2, H], F16)
            nc.sync.dma_start(out=wout, in_=woutT_d.rearrange("(q p) e -> p q e", p=128))
            ident = sbW.tile([128, 128], F16)
            nc.sync.dma_start(out=ident, in_=ident_d[:, :])

            # persistent per-chunk q/k tiles, per-j-tile v tiles
            qTc = [[sbA.tile([128, 512], F32R, tag=f"qT{p}_{i}", name=f"qT{p}_{i}")
                    for i in range(IC)] for p in range(2)]
            kTc = [[sbA.tile([128, 512], F32R, tag=f"kT{p}_{i}", name=f"kT{p}_{i}")
                    for i in range(IC)] for p in range(2)]
            vtj = [sbA.tile([128, HEADS_PER_CORE, 65], F16, tag=f"vt{jt}",
                            name=f"vt{jt}") for jt in range(JT)]
            # ones column of v_aug, written once (on idle GPSIMD)
            for jt in range(JT):
                nc.gpsimd.memset(vtj[jt][:, :, 64:65], 1.0)
            o2T = [[sbA.tile([128, 512], F16, tag=f"o2T{p}_{ic}",
                             name=f"o2T{p}_{ic}")
                    for ic in range(IC)] for p in range(2)]

            # ---- chain emitters ------------------------------------------
            def q_mm(p, ic, ps, k):
                nc.tensor.matmul(ps, lhsT=wq[:, p, k, :],
                                 rhs=xt[ic][:, k, :],
                                 start=(k == 0), stop=(k == KT - 1))

            def k_mm(p, c, ps, k):
                nc.tensor.matmul(ps, lhsT=wk[:, p, k, :],
                                 rhs=xt[c][:, k, :],
                                 start=(k == 0), stop=(k == KT - 1))

            def q_fin(p, ic, ps):
                nc.vector.tensor_scalar_add(qTc[p][ic], ps, bqk_sb[:, p:p + 1])

            def k_fin(p, c, ps):
                nc.vector.tensor_scalar_add(kTc[p][c], ps, bqk_sb[:, 2 + p:3 + p])

            def chain_parts(kind, p, i, nparts=4):
                state = {}
                per = KT // nparts
                mm = q_mm if kind == "q" else k_mm
                fin = q_fin if kind == "q" else k_fin

                def part(j):
                    def f():
                        if j == 0:
                            state["ps"] = psCH.tile([128, 512], F32, tag="ch",
                                                    name=f"ps_{kind}")
                        for k in range(per * j, per * (j + 1)):
                            mm(p, i, state["ps"], k)
                        if j == nparts - 1:
                            fin(p, i, state["ps"])
                    return f
                return [part(j) for j in range(nparts)]

            def whole_chain(kind, p, i):
                for f in chain_parts(kind, p, i, nparts=1):
                    f()

            def v_chain(jt, vp):
                # v projection for ONE head pair: halves the v work the
                # first (PE-oversubscribed) chunk must absorb; pair-1's
                # chains ride the ACT-bound middle chunks' spare PE.
                c, jl = jt // 4, jt % 4
                ps = psCH.tile([128, 128], F32, tag="ch", name="ps_v")
                for k in range(KT):
                    nc.tensor.matmul(ps,
                                     lhsT=xt[c][:, k, 128 * jl:128 * jl + 128],
                                     rhs=wv[:, vp, k, :],
                                     start=(k == 0), stop=(k == KT - 1))
                nc.vector.tensor_add(
                    vtj[jt][:, 2 * vp:2 * vp + 2, 0:64],
                    ps.rearrange("p (h d) -> p h d", h=2),
                    bvrep.rearrange("p (h d) -> p h d",
                                    h=HEADS_PER_CORE)[:, 2 * vp:2 * vp + 2, :])

            def s_pair(p, ic, jt):
                c, jl = jt // 4, jt % 4
                st = psST.tile([128, 1024], F32, tag="st", name="st")
                nc.tensor.matmul(st[:, 0:512],
                                 lhsT=kTc[p][c][0:64, 128 * jl:128 * jl + 128],
                                 rhs=qTc[p][ic][0:64, :],
                                 start=True, stop=True)
                nc.tensor.matmul(st[:, 512:1024],
                                 lhsT=kTc[p][c][64:128, 128 * jl:128 * jl + 128],
                                 rhs=qTc[p][ic][64:128, :],
                                 start=True, stop=True)
                return st

            def pv_step(p, jt, pt, accs):
                for t in range(4):
                    for h in range(2):
                        o0 = 130 * (t % 2) + 65 * h
                        # start=True zeroes the whole 2KB PSUM bank
                        # (ZERO_REGION_SIZE): only the first matmul per acc
                        # tile may set it; later regions' first writes land
                        # on pending-zero bytes and initialize correctly.
                        nc.tensor.matmul(
                            accs[t // 2][:, o0:o0 + 65],
                            lhsT=pt[:, 512 * h + 128 * t:512 * h + 128 * t + 128],
                            rhs=vtj[jt][:, 2 * p + h, :],
                            start=(jt == 0 and t % 2 == 0 and h == 0),
                            stop=(jt == JT - 1 and t % 2 == 1 and h == 1),
                            skip_group_check=True)

            def attention_body(p, ic, fillers=None, st_in=None, nxt=None):
                # acc regions: accA holds q-subtiles 0,1; accB holds 2,3.
                # Region (t, h) = [:, 130*(t%2) + 65*h :][:65]; col 64 is z.
                accA = psACC.tile([128, 260], F32, tag="accA", name="accA")
                accB = psACC.tile([128, 260], F32, tag="accB", name="accB")
                accs = [accA, accB]

                st = st_in if st_in is not None else s_pair(p, ic, 0)
                st_out = None
                pts = [None, None]  # PV trails exp by one step
                for jt in range(JT):
                    if fillers:
                        for f in fillers.get(jt, ()):
                            f()
                    if jt + 1 < JT:
                        st_next = s_pair(p, ic, jt + 1)
                    elif nxt is not None:
                        st_next = st_out = s_pair(nxt[0], nxt[1], 0)
                    else:
                        st_next = None
                    pt = sbPT.tile([128, 1024], F16, tag="pt", name="pt")
                    nc.scalar.activation(pt, st, EXP, scale=float(HD) ** -0.5)
                    pts[jt % 2] = pt
                    if jt >= 1:
                        pv_step(p, jt - 1, pts[(jt - 1) % 2], accs)
                    st = st_next
                pv_step(p, JT - 1, pts[(JT - 1) % 2], accs)
                return (p, ic, accs), st_out

            def norm_mul(o2sb, accs, a, zrec):
                # One broadcast multiply per acc bank: o2sb[q, t, (h c)] =
                # acc regions * (1/z) with z stride-0-broadcast along c.
                out_v = o2sb[:, 2 * a:2 * a + 2, :].rearrange(
                    "p t (h c) -> p (t h) c", c=64)
                acc_v = accs[a].rearrange("p (r c) -> p r c", c=65)[:, :, 0:64]
                z_v = zrec[:, 4 * a:4 * a + 4].rearrange(
                    "p (r one) -> p r one", one=1)
                acc_b, z_b = bass.broadcast_tensor_aps(acc_v, z_v)
                nc.vector.tensor_mul(out_v, acc_b, z_b)

            def normalize_fin(pend):
                # o2sb[q, t, d2] = acc / z (per-partition scalars), then a
                # DMA-engine transpose flips to o2T[d2, t*128+q].
                p, ic, accs = pend
                zrec = sbZ.tile([128, 8], F32, tag="zrec", name="zrec")
                o2sb = sbZ.tile([128, 4, 128], F16, tag="o2sb", name="o2sb")
                # per-acc interleave: accA's slot releases after two ops so
                # the next chunk's first PV (WAR on that slot) starts sooner
                for a in range(2):
                    nc.vector.reciprocal(
                        zrec[:, 4 * a:4 * a + 4],
                        accs[a].rearrange("p (r c) -> p r c", c=65)[:, :, 64])
                    norm_mul(o2sb, accs, a, zrec)
                nc.sync.dma_start_transpose(
                    out=o2T[p][ic].rearrange("p (t q) -> p t q", t=4),
                    in_=o2sb.rearrange("p t q -> p (t q)"))

            def proj_unit(ic, itl, ec, ost_box, tail=False):
                t0 = 512 * ic + 128 * itl
                # in the tail the score banks are free: borrow them for a
                # deeper psum rotation so copies never pace the projections
                pool, tg = (psST, "st") if (tail and ec == 1) else (psCH, "ch")
                ps = pool.tile([128, 512], F32, tag=tg, name="ps_o")
                e0 = 512 * ec
                nc.tensor.matmul(ps,
                                 lhsT=o2T[0][ic][:, 128 * itl:128 * itl + 128],
                                 rhs=wout[:, 0, e0:e0 + 512],
                                 start=True, stop=False)
                nc.tensor.matmul(ps,
                                 lhsT=o2T[1][ic][:, 128 * itl:128 * itl + 128],
                                 rhs=wout[:, 1, e0:e0 + 512],
                                 start=False, stop=True)
                if ec == 0:
                    ost_box["t"] = sbOut.tile([128, 1024], F16, tag="ost",
                                              name="ost")
                ost = ost_box["t"]
                # One full-row store per itl halves the serialized HWDGE
                # holds.  In the tail the Scalar engine is idle: give it
                # half the PSUM->SBUF copies so DVE isn't the pacer.
                if tail and ec == 0:
                    nc.scalar.copy(ost[:, e0:e0 + 512], ps)
                else:
                    nc.vector.tensor_copy(ost[:, e0:e0 + 512], ps)
                if ec == 1:
                    nc.sync.dma_start(out=out_d[t0:t0 + 128, :], in_=ost)

            def proj_units(ic, itl, tail=False):
                box = {}
                return [
                    (lambda i, t, e, b: lambda: proj_unit(i, t, e, b, tail))(
                        ic, itl, ec, box)
                    for ec in range(2)
                ]

            # ---- emission order = engine execution order ------------------
            # PE p-state warmup: a dense stream of trivial matmuls keeps the
            # tensor engine continuously busy from ~0.3us (a DVE memset
            # seeds the operand, no DMA dependency) until the first real
            # chains (~10us, gated on the x chunk-0 DMA), so the ramp to
            # the 2.4GHz p-state is paid on throwaway work.
            wuseed = sbZ.tile([128, 2], F16, tag="wus", name="wus")
            nc.vector.memset(wuseed[:, :], 0.0)
            wups = psCH.tile([128, 16], F32, tag="ch", name="wu")
            for _ in range(880):
                nc.tensor.matmul(wups[0:2, 0:2], lhsT=wuseed[:, 0:2],
                                 rhs=wuseed[:, 0:2], start=True, stop=True,
                                 skip_group_check=True)
            # interleaved first chains: q and k advance together per k-tile
            # (subtile deps let each matmul start as its half-DMA lands)
            ps_q0 = psCH.tile([128, 512], F32, tag="ch", name="ps_q0")
            ps_k0 = psCH.tile([128, 512], F32, tag="ch", name="ps_k0")
            for k in range(KT):
                q_mm(0, 0, ps_q0, k)
                k_mm(0, 0, ps_k0, k)
            q_fin(0, 0, ps_q0)
            # split the first k bias-add: s(0) only reads keys 0-127, so it
            # can start after the first half lands
            nc.vector.tensor_scalar_add(kTc[0][0][:, 0:256], ps_k0[:, 0:256],
                                        bqk_sb[:, 2:3])
            nc.vector.tensor_scalar_add(kTc[0][0][:, 256:512], ps_k0[:, 256:512],
                                        bqk_sb[:, 2:3])

            segs = []
            f00 = {jt: [] for jt in range(JT)}
            for jt in range(JT):
                f00[max(0, jt - 1)].append((lambda j: lambda: v_chain(j, 0))(jt))
            for ci in range(1, IC):
                for j, f in enumerate(chain_parts("k", 0, ci, nparts=2)):
                    f00[4 * ci - 4 + 1 + j].append(f)
            for j, f in enumerate(chain_parts("q", 0, 1, nparts=2)):
                f00[13 + j].append(f)
            segs.append((0, 0, f00))

            # Filler load balancing: every middle seg stays under the
            # 16.6us ACT pace (6.3us filler budget).  q(1,ic) is only
            # needed by the END of seg (1,ic-1) (score handover), so those
            # chains ride late segs; pair-1 v-chains spread over segs 1-4.
            for ic in range(1, IC):
                fq = {}
                if ic == 1:
                    for j, f in enumerate(chain_parts("k", 1, 0)):
                        fq.setdefault(1 + j, []).append(f)
                    for j, f in enumerate(chain_parts("q", 1, 0)):
                        fq.setdefault(5 + j, []).append(f)
                    for j, vj in enumerate((0, 1)):
                        fq.setdefault(6 * j, []).append(
                            (lambda x: lambda: v_chain(x, 1))(vj))
                elif ic == 2:
                    for j, f in enumerate(chain_parts("k", 1, 1)):
                        fq.setdefault(1 + j, []).append(f)
                    for j, vj in enumerate((2, 3, 4, 5)):
                        fq.setdefault((0, 5, 6, 7)[j], []).append(
                            (lambda x: lambda: v_chain(x, 1))(vj))
                else:
                    for j, f in enumerate(chain_parts("k", 1, 2)):
                        fq.setdefault(1 + j, []).append(f)
                    for j, vj in enumerate((6, 7, 8, 9, 10, 11)):
                        fq.setdefault((0, 5, 6, 7, 8, 9)[j], []).append(
                            (lambda x: lambda: v_chain(x, 1))(vj))
                if ic < IC - 1:
                    for j, f in enumerate(chain_parts("q", 0, ic + 1)):
                        fq.setdefault(10 + j, []).append(f)
                segs.append((0, ic, fq))
            for ic in range(IC):
                fp = {}
                if ic == 0:
                    for j, vj in enumerate((12, 13, 14, 15)):
                        fp.setdefault((0, 1, 11, 12)[j], []).append(
                            (lambda x: lambda: v_chain(x, 1))(vj))
                    for j, f in enumerate(chain_parts("q", 1, 1)):
                        fp.setdefault(2 + j, []).append(f)
                    for j, f in enumerate(chain_parts("k", 1, 3)):
                        fp.setdefault(7 + j, []).append(f)
                else:
                    units = [u for itl in range(4) for u in proj_units(ic - 1, itl)]
                    for u, f in zip((4, 5, 6, 7, 8, 9, 11, 12), units):
                        fp.setdefault(u, []).append(f)
                    if ic < IC - 1:
                        for j, f in enumerate(chain_parts("q", 1, ic + 1)):
                            fp.setdefault(j, []).append(f)
                segs.append((1, ic, fp))

            st_hand = None
            pend = None
            for idx, (p, ic, fillers) in enumerate(segs):
                nxt = segs[idx + 1][:2] if idx + 1 < len(segs) else None
                pend, st_hand = attention_body(p, ic, fillers,
                                               st_in=st_hand, nxt=nxt)
                if idx + 1 < len(segs):
                    normalize_fin(pend)
            # tail: pipeline the last chunk per q-subtile so each subtile's
            # normalize -> transpose -> projection -> store chain starts as
            # soon as possible instead of after a monolithic transpose.
            p, ic, accs = pend
            zrec = sbZ.tile([128, 8], F32, tag="zrec", name="zrec")
            for a in range(2):
                nc.vector.reciprocal(
                    zrec[:, 4 * a:4 * a + 4],
                    accs[a].rearrange("p (r c) -> p r c", c=65)[:, :, 64])
            o2sb = sbZ.tile([128, 4, 128], F16, tag="o2sb", name="o2sb")
            for t in range(4):
                if t % 2 == 0:
                    # one mul per acc bank (covers two subtiles): shorter DVE
                    # chain, so the transposes start sooner
                    a = t // 2
                    out_v = o2sb[:, 2 * a:2 * a + 2, :].rearrange(
                        "p t (h c) -> p (t h) c", c=64)
                    acc_v = accs[a].rearrange("p (r c) -> p r c", c=65)[:, :, 0:64]
                    z_v = zrec[:, 4 * a:4 * a + 4].rearrange(
                        "p (r one) -> p r one", one=1)
                    acc_b, z_b = bass.broadcast_tensor_aps(acc_v, z_v)
                    nc.vector.tensor_mul(out_v, acc_b, z_b)
                # PE-side transpose: ~1us lower latency than the DMA path,
                # and the PE is idle in the tail anyway.  The transposes
                # borrow the acc banks (free after the norm reads) so they
                # don't rotate through the proj PSUM slots.
                pst = psACC.tile([128, 128], F16, tag="accA" if t % 2 == 0 else "accB",
                                 name="pst")
                nc.tensor.transpose(pst, o2sb[:, t, :], ident)
                # ACT has slack in the tail; keep the tiny proj-gating copy
                # off the DVE queue (behind the big ost copies)
                nc.scalar.copy(o2T[p][ic][:, 128 * t:128 * (t + 1)], pst)
                for f in proj_units(IC - 1, t, tail=True):
                    f()
    nc.compile()
    return nc


def _get_nc():
    if "nc" not in _CACHE:
        _CACHE["nc"] = _build()
    return _CACHE["nc"]


def make_in_maps(x, w_qkv, b_qkv, w_out):
    """Per-core input dicts for the SPMD kernel."""
    f32 = np.float32
    x = np.asarray(x, f32)
    w_qkv = np.asarray(w_qkv, f32)
    b_qkv = np.asarray(b_qkv, f32)
    w_out = np.asarray(w_out, f32)
    in_maps = []
    for c in range(N_CORES):
        b = c // 4
        g = c % 4
        r0, r1 = 64 * 4 * g, 64 * 4 * (g + 1)
        in_maps.append({
            "xT": np.ascontiguousarray(x[b].T).astype(np.float16),
            "wqT": np.ascontiguousarray(
                w_qkv[r0:r1, :].T.reshape(8, 128, 2, 128).transpose(1, 2, 0, 3)
                .reshape(128, 2048)).astype(np.float16),
            "wkT": np.ascontiguousarray(
                w_qkv[H + r0:H + r1, :].T.reshape(8, 128, 2, 128)
                .transpose(1, 2, 0, 3).reshape(128, 2048)).astype(np.float16),
            "wvT": np.ascontiguousarray(
                w_qkv[2 * H + r0:2 * H + r1, :].T.reshape(8, 128, 2, 128)
                .transpose(1, 2, 0, 3).reshape(128, 2048)).astype(np.float16),
            "woutT": np.ascontiguousarray(w_out[:, r0:r1].T).astype(np.float16),
            "bqk": np.concatenate([
                b_qkv[r0:r1].reshape(2, 128).T,
                b_qkv[H + r0:H + r1].reshape(2, 128).T], axis=1).copy(),
            "bvrep": np.tile(b_qkv[2 * H + r0:2 * H + r1][None, :], (128, 1)),
            "ident": np.eye(128, dtype=np.float16),
        })
    return in_maps


def assemble(results, b_out):
    """Sum per-core partials into the full [B, L, H] output."""
    out = np.empty((B, L, H), np.float32)
    for b in range(B):
        acc = results[4 * b]["out"].astype(np.float32)
        for c in range(4 * b + 1, 4 * b + 4):
            acc = acc + results[c]["out"]
        out[b] = acc + np.asarray(b_out, np.float32)[None, :]
    return out


def kernel(x, w_qkv, b_qkv, w_out, b_out):
    from concourse.bass_utils import run_bass_kernel_spmd

    nc = _get_nc()
    in_maps = make_in_maps(x, w_qkv, b_qkv, w_out)
    res = run_bass_kernel_spmd(nc, in_maps, core_ids=list(range(N_CORES)))
    return assemble(res.results, b_out)


if __name__ == "__main__":
    rng = np.random.default_rng(0)
    x = rng.standard_normal((B, L, H), dtype=np.float32)
    w_qkv = (rng.standard_normal((3 * H, H), dtype=np.float32) / np.sqrt(H)).astype(np.float32)
    b_qkv = (rng.standard_normal(3 * H).astype(np.float32) * 0.01)
    w_out = (rng.standard_normal((H, H), dtype=np.float32) / np.sqrt(H)).astype(np.float32)
    b_out = (rng.standard_normal(H).astype(np.float32) * 0.01)
    out = kernel(x, w_qkv, b_qkv, w_out, b_out)
    print("kernel output", out.shape, out.dtype)

